# revision 5
# baseline (speedup 1.0000x reference)
"""Trainium2 Bass kernel for nn_Match_Decoder (GRU step + Luong attention + vocab projection).

Strategy (8-core SPMD):
  - Host precomputes x = [emb[word], last_context] (embedding row gather) and
    transposes/permutes weights so every matvec runs on the PE with the big
    operand streaming as rhs from its natural DMA layout.
  - GRU: gx/gh row-sharded 387 rows/core (gate-aligned 129-chunks), AllGather
    (774 floats/core), every core computes gates redundantly on an [8,129] layout.
  - Attention: W_a replicated (rows permuted to match the h column layout), q and
    per-core partial softmax stats + context partials computed locally over a
    1024-step encoder shard; second AllGather (1034 floats/core) combines them.
  - Vocab projection: W_out columns permuted to the joined = [h; ctx] on-chip
    column layout, transposed to [2064, 6400] per core (vocab-sharded, padded),
    streamed as 18 row-chunk tiles; logits accumulate in PSUM as 13 x [1, <=512]
    rows packed 3-per-bank at partitions {0,32,64}.
  - Final log_softmax normalization on host (subtract scalar logsumexp).
"""

import os
import numpy as np

V = 50257
H = 1032
IN = 2 * H          # 2064
TH = 3 * H          # 3096
S = 8192
NCORES = 8
GC = TH // NCORES   # 387 gate rows per core
HC = H // NCORES    # 129
SEQC = S // NCORES  # 1024
VC = 6400           # padded vocab rows per core (50 tiles of 128)
NVB = 13            # vocab blocks: 12x512 + 1x256
VROWS = [6272] * 7 + [6353]
V0 = [6272 * i for i in range(7)] + [43904]

F32 = np.float32

LAST_EXEC_TIME_NS = None
_NC_CACHE = {}


def _vb_n(vb):
    return 512 if vb < 12 else 256


def _build_nc():
    import concourse.mybir as mybir
    import concourse.tile as tile
    from concourse import bacc
    from concourse.masks import make_identity

    fp = mybir.dt.float32
    nc = bacc.Bacc("TRN2", target_bir_lowering=False, debug=False, num_devices=NCORES)

    # ---- per-core DRAM inputs ----
    wih_t = nc.dram_tensor("wih_t", [IN + 1, GC], fp, kind="ExternalInput")
    whh_t = nc.dram_tensor("whh_t", [H + 1, GC], fp, kind="ExternalInput")
    wa_p = nc.dram_tensor("wa_p", [H, H], fp, kind="ExternalInput")
    enc_c = nc.dram_tensor("enc_c", [SEQC, H], fp, kind="ExternalInput")
    wout_t = nc.dram_tensor("wout_t", [IN, VC], fp, kind="ExternalInput")
    b_c = nc.dram_tensor("b_c", [128, 5 * 512], fp, kind="ExternalInput")
    x_cols = nc.dram_tensor("x_cols", [128, 17], fp, kind="ExternalInput")
    hp_cols = nc.dram_tensor("hp_cols", [128, 9], fp, kind="ExternalInput")
    hp829 = nc.dram_tensor("hp829", [8, HC], fp, kind="ExternalInput")
    oh = nc.dram_tensor("oh", [8, 1], fp, kind="ExternalInput")

    # ---- per-core DRAM outputs ----
    logits_o = nc.dram_tensor("logits_o", [VC], fp, kind="ExternalOutput")
    h_o = nc.dram_tensor("h_o", [H], fp, kind="ExternalOutput")
    ctx_o = nc.dram_tensor("ctx_o", [H], fp, kind="ExternalOutput")
    w_o = nc.dram_tensor("w_o", [SEQC], fp, kind="ExternalOutput")

    ax = mybir.AxisListType.X
    AF = mybir.ActivationFunctionType

    with tile.TileContext(nc) as tc:
        with (
            tc.tile_pool(name="const", bufs=1) as const,
            tc.tile_pool(name="sm", bufs=1) as sm,          # small long-lived sbuf
            tc.tile_pool(name="att", bufs=1) as att,        # W_a + enc
            tc.tile_pool(name="psw", bufs=1, space="PSUM") as psw,
            tc.tile_pool(name="mps", bufs=3, space="PSUM") as mps,
            tc.tile_pool(name="dram", bufs=1, space="DRAM") as dram,
        ):
            ident = const.tile([128, 128], fp)
            make_identity(nc, ident)
            ones = const.tile([128, 128], fp)
            nc.vector.memset(ones, 1.0)

            # ---------- stage 1: GRU matvecs gx, gh ----------
            x_sb = sm.tile([128, 17], fp)
            nc.sync.dma_start(out=x_sb, in_=x_cols.ap())
            hpc_sb = sm.tile([128, 9], fp)
            nc.sync.dma_start(out=hpc_sb, in_=hp_cols.ap())
            hp8_sb = sm.tile([8, HC], fp)
            nc.sync.dma_start(out=hp8_sb, in_=hp829.ap())
            oh_sb = sm.tile([8, 1], fp)
            nc.sync.dma_start(out=oh_sb, in_=oh.ap())
            b_sb = sm.tile([128, 5 * 512], fp)
            nc.sync.dma_start(out=b_sb, in_=b_c.ap())

            gx_ps = mps.tile([1, 512], fp, tag="m", name="gx_ps")
            gh_ps = mps.tile([1, 512], fp, tag="m", name="gh_ps")
            with tc.tile_pool(name="gru", bufs=1) as gru:
                wih_sb = gru.tile([128, 16 * GC], fp)
                nc.sync.dma_start(
                    out=wih_sb.rearrange("p (t r) -> p t r", t=16),
                    in_=wih_t.ap()[0:2048, :].rearrange("(t p) r -> p t r", p=128)
                )
                wih_st = gru.tile([17, GC], fp)
                nc.sync.dma_start(out=wih_st, in_=wih_t.ap()[2048 : IN + 1, :])
                whh_sb = gru.tile([128, 8 * GC], fp)
                nc.sync.dma_start(
                    out=whh_sb.rearrange("p (t r) -> p t r", t=8),
                    in_=whh_t.ap()[0:1024, :].rearrange("(t p) r -> p t r", p=128)
                )
                whh_st = gru.tile([9, GC], fp)
                nc.sync.dma_start(out=whh_st, in_=whh_t.ap()[1024 : H + 1, :])

                for t in range(16):
                    nc.tensor.matmul(
                        gx_ps[0:1, 0:GC], x_sb[:, t : t + 1],
                        wih_sb[:, GC * t : GC * (t + 1)], start=(t == 0), stop=False,
                    )
                nc.tensor.matmul(
                    gx_ps[0:1, 0:GC], x_sb[0:17, 16:17], wih_st[0:17, :], start=False, stop=True
                )
                for t in range(8):
                    nc.tensor.matmul(
                        gh_ps[0:1, 0:GC], hpc_sb[:, t : t + 1],
                        whh_sb[:, GC * t : GC * (t + 1)], start=(t == 0), stop=False,
                    )
                nc.tensor.matmul(
                    gh_ps[0:1, 0:GC], hpc_sb[0:9, 8:9], whh_st[0:9, :], start=False, stop=True
                )
            pay1 = sm.tile([1, 2 * GC], fp)
            nc.scalar.copy(pay1[0:1, 0:GC], gx_ps[0:1, 0:GC])
            nc.scalar.copy(pay1[0:1, GC : 2 * GC], gh_ps[0:1, 0:GC])

            cc1_in = dram.tile([1, 2 * GC], fp)
            cc1_out = dram.tile([NCORES, 2 * GC], fp)
            nc.gpsimd.dma_start(out=cc1_in, in_=pay1)
            nc.gpsimd.collective_compute(
                "AllGather", mybir.AluOpType.bypass,
                replica_groups=[list(range(NCORES))],
                ins=[cc1_in.opt()], outs=[cc1_out.opt()],
            )
            g1 = sm.tile([8, 2 * GC], fp)
            nc.gpsimd.dma_start(out=g1, in_=cc1_out)

            # ---------- stage 2: gates on [8, 129] ----------
            xr, xz, xn = (g1[:, 129 * k : 129 * (k + 1)] for k in range(3))
            hr, hz, hn = (g1[:, GC + 129 * k : GC + 129 * (k + 1)] for k in range(3))
            t_r = sm.tile([8, HC], fp)
            nc.vector.tensor_add(t_r, xr, hr)
            r_g = sm.tile([8, HC], fp)
            nc.scalar.activation(r_g, t_r, AF.Sigmoid)
            t_z = sm.tile([8, HC], fp)
            nc.vector.tensor_add(t_z, xz, hz)
            z_g = sm.tile([8, HC], fp)
            nc.scalar.activation(z_g, t_z, AF.Sigmoid)
            t_n = sm.tile([8, HC], fp)
            nc.vector.tensor_mul(t_n, r_g, hn)
            nc.vector.tensor_add(t_n, t_n, xn)
            n_g = sm.tile([8, HC], fp)
            nc.scalar.activation(n_g, t_n, AF.Tanh)
            # h_new = n + z*(hp - n)
            h_new = sm.tile([8, HC], fp)
            nc.vector.tensor_sub(h_new, hp8_sb, n_g)
            nc.vector.tensor_mul(h_new, z_g, h_new)
            nc.vector.tensor_add(h_new, n_g, h_new)
            nc.gpsimd.dma_start(out=h_o.ap().rearrange("(i j) -> i j", i=8), in_=h_new)

            # ---------- stage 3: J_h columns ----------
            hT_ps = mps.tile([128, 8], fp, tag="m", name="hT_ps")
            nc.tensor.transpose(hT_ps, h_new[:, 0:128], ident[0:8, 0:8])
            hT = sm.tile([128, 8], fp)
            nc.scalar.copy(hT, hT_ps)
            h_strag = h_new[0:8, 128:129]  # K=8 column

            # ---------- stage 4: q = h @ W_a (permuted rows) ----------
            wa_sb = att.tile([128, 8 * H], fp)
            nc.sync.dma_start(
                out=wa_sb.rearrange("p (t l) -> p t l", t=8),
                in_=wa_p.ap()[0:1024, :].rearrange("(t p) l -> p t l", p=128)
            )
            wa_st = att.tile([8, H], fp)
            nc.sync.dma_start(out=wa_st, in_=wa_p.ap()[1024:H, :])

            q_row = sm.tile([1, H], fp)
            for c0, nn in ((0, 512), (512, 512), (1024, 8)):
                qp = mps.tile([1, 512], fp, tag="m", name=f"qp{c0}")
                for m in range(8):
                    nc.tensor.matmul(
                        qp[0:1, 0:nn], hT[:, m : m + 1],
                        wa_sb[:, H * m + c0 : H * m + c0 + nn], start=(m == 0), stop=False,
                    )
                nc.tensor.matmul(
                    qp[0:1, 0:nn], h_strag, wa_st[0:8, c0 : c0 + nn], start=False, stop=True
                )
                nc.scalar.copy(q_row[0:1, c0 : c0 + nn], qp[0:1, 0:nn])
            q_rep = sm.tile([128, H], fp)
            for c0, nn in ((0, 512), (512, 512), (1024, 8)):
                qr_ps = mps.tile([128, 512], fp, tag="m", name=f"qr{c0}")
                nc.tensor.matmul(
                    qr_ps[:, 0:nn], ones[0:1, 0:128], q_row[0:1, c0 : c0 + nn],
                    start=True, stop=True,
                )
                nc.scalar.copy(q_rep[:, c0 : c0 + nn], qr_ps[:, 0:nn])

            # ---------- stage 5: scores + local softmax stats ----------
            enc_sb = att.tile([128, 8 * H], fp)
            nc.sync.dma_start(
                out=enc_sb.rearrange("p (t l) -> p t l", t=8),
                in_=enc_c.ap().rearrange("(t p) l -> p t l", p=128)
            )
            s_sb = sm.tile([128, 8], fp)
            tmp = sm.tile([128, H], fp)
            for t in range(8):
                nc.vector.tensor_mul(tmp, enc_sb[:, H * t : H * (t + 1)], q_rep)
                nc.vector.reduce_sum(s_sb[:, t : t + 1], tmp, axis=ax)
            m_p = sm.tile([128, 1], fp)
            nc.vector.reduce_max(m_p, s_sb, axis=ax)
            mT_ps = mps.tile([1, 128], fp, tag="m", name="mT_ps")
            nc.tensor.transpose(mT_ps, m_p, ident)
            mT = sm.tile([1, 128], fp)
            nc.scalar.copy(mT, mT_ps)
            m_i = sm.tile([1, 1], fp)
            nc.vector.reduce_max(m_i, mT, axis=ax)
            negm = sm.tile([1, 1], fp)
            nc.vector.tensor_scalar_mul(negm, m_i, -1.0)
            nb_ps = mps.tile([128, 1], fp, tag="m", name="nb_ps")
            nc.tensor.matmul(nb_ps, ones[0:1, 0:128], negm, start=True, stop=True)
            negb = sm.tile([128, 1], fp)
            nc.scalar.copy(negb, nb_ps)
            w_un = sm.tile([128, 8], fp)
            e_p = sm.tile([128, 1], fp)
            nc.scalar.activation(w_un, s_sb, AF.Exp, bias=negb, accum_out=e_p)
            ei_ps = mps.tile([1, 1], fp, tag="m", name="ei_ps")
            nc.tensor.matmul(ei_ps, ones[0:128, 0:1], e_p, start=True, stop=True)

            # ---------- stage 6: context partial + collective 2 ----------
            pay2 = sm.tile([1, H + 2], fp)
            for c0, nn in ((0, 512), (512, 512), (1024, 8)):
                cp = mps.tile([1, 512], fp, tag="m", name=f"cp{c0}")
                for t in range(8):
                    nc.tensor.matmul(
                        cp[0:1, 0:nn], w_un[:, t : t + 1],
                        enc_sb[:, H * t + c0 : H * t + c0 + nn],
                        start=(t == 0), stop=(t == 7),
                    )
                nc.scalar.copy(pay2[0:1, c0 : c0 + nn], cp[0:1, 0:nn])
            nc.scalar.copy(pay2[0:1, H : H + 1], m_i)
            nc.scalar.copy(pay2[0:1, H + 1 : H + 2], ei_ps)

            cc2_in = dram.tile([1, H + 2], fp)
            cc2_out = dram.tile([NCORES, H + 2], fp)
            nc.gpsimd.dma_start(out=cc2_in, in_=pay2)
            nc.gpsimd.collective_compute(
                "AllGather", mybir.AluOpType.bypass,
                replica_groups=[list(range(NCORES))],
                ins=[cc2_in.opt()], outs=[cc2_out.opt()],
            )
            g2 = sm.tile([8, H + 2], fp)
            nc.gpsimd.dma_start(out=g2, in_=cc2_out)

            # ---------- stage 8a: W_out h-phase (emitted before combine for PE order) ----------
            pst = [psw.tile([128, 512], fp, tag=f"ps{g}", name=f"ps{g}") for g in range(5)]
            ring_ctx = tc.tile_pool(name="ring", bufs=2)
            ring = ring_ctx.__enter__()

            def w_phase(chunks, first, last):
                for t, (r0, k, jcol) in enumerate(chunks):
                    wt = ring.tile([128, VC], fp, tag="w", name=f"wt{r0}")
                    nc.sync.dma_start(out=wt[0:k, :], in_=wout_t.ap()[r0 : r0 + k, :])
                    for vb in range(NVB):
                        n = _vb_n(vb)
                        pt, row = pst[vb // 3], 32 * (vb % 3)
                        nc.tensor.matmul(
                            pt[row : row + 1, 0:n], jcol()[0:k, :],
                            wt[0:k, 512 * vb : 512 * vb + n],
                            start=(first and t == 0), stop=(last and t == len(chunks) - 1),
                        )

            h_chunks = [(128 * t, 128, (lambda t=t: hT[:, t : t + 1])) for t in range(8)]
            h_chunks.append((1024, 8, lambda: h_strag))
            w_phase(h_chunks, first=True, last=False)

            # ---------- stage 7: cross-core softmax combine ----------
            mcol = g2[0:8, H : H + 1]
            ecol = g2[0:8, H + 1 : H + 2]
            mT2_ps = mps.tile([1, 8], fp, tag="m", name="mT2_ps")
            nc.tensor.transpose(mT2_ps, mcol, ident[0:8, 0:8])
            mT2 = sm.tile([1, 8], fp)
            nc.scalar.copy(mT2, mT2_ps)
            M_i = sm.tile([1, 1], fp)
            nc.vector.reduce_max(M_i, mT2, axis=ax)
            negM = sm.tile([1, 1], fp)
            nc.vector.tensor_scalar_mul(negM, M_i, -1.0)
            n8_ps = mps.tile([8, 1], fp, tag="m", name="n8_ps")
            nc.tensor.matmul(n8_ps, ones[0:1, 0:8], negM, start=True, stop=True)
            negM8 = sm.tile([8, 1], fp)
            nc.scalar.copy(negM8, n8_ps)
            expm = sm.tile([8, 1], fp)
            nc.scalar.activation(expm, mcol, AF.Exp, bias=negM8)
            prod = sm.tile([8, 1], fp)
            nc.vector.tensor_mul(prod, ecol, expm)
            tot_ps = mps.tile([1, 1], fp, tag="m", name="tot_ps")
            nc.tensor.matmul(tot_ps, ones[0:8, 0:1], prod, start=True, stop=True)
            tot = sm.tile([1, 1], fp)
            nc.scalar.copy(tot, tot_ps)
            invt = sm.tile([1, 1], fp)
            nc.vector.reciprocal(invt, tot)
            scaled = sm.tile([8, H], fp)
            nc.vector.tensor_scalar_mul(scaled, g2[0:8, 0:H], expm)
            ctx_un = sm.tile([1, H], fp)
            for c0, nn in ((0, 512), (512, 512), (1024, 8)):
                cr_ps = mps.tile([1, 512], fp, tag="m", name=f"cr{c0}")
                nc.tensor.matmul(
                    cr_ps[0:1, 0:nn], ones[0:8, 0:1], scaled[:, c0 : c0 + nn],
                    start=True, stop=True,
                )
                nc.scalar.copy(ctx_un[0:1, c0 : c0 + nn], cr_ps[0:1, 0:nn])
            ctx_row = sm.tile([1, H], fp)
            nc.vector.tensor_scalar_mul(ctx_row, ctx_un, invt)
            nc.gpsimd.dma_start(out=ctx_o.ap().rearrange("(one h) -> one h", one=1), in_=ctx_row)

            # J_c columns via transposes of ctx_row
            jc_ps = mps.tile([128, 9], fp, tag="m", name="jc_ps")
            for u in range(8):
                nc.tensor.transpose(
                    jc_ps[:, u : u + 1], ctx_row[0:1, 128 * u : 128 * (u + 1)],
                    ident[0:1, 0:1],
                )
            nc.tensor.transpose(jc_ps[0:8, 8:9], ctx_row[0:1, 1024:H], ident[0:1, 0:1])
            jc = sm.tile([128, 9], fp)
            nc.scalar.copy(jc, jc_ps)

            # attn weights normalize + output
            me_ps = mps.tile([1, 1], fp, tag="m", name="me_ps")
            nc.tensor.matmul(me_ps, oh_sb, expm, start=True, stop=True)
            me = sm.tile([1, 1], fp)
            nc.scalar.copy(me, me_ps)
            fac = sm.tile([1, 1], fp)
            nc.vector.tensor_mul(fac, me, invt)
            fb_ps = mps.tile([128, 1], fp, tag="m", name="fb_ps")
            nc.tensor.matmul(fb_ps, ones[0:1, 0:128], fac, start=True, stop=True)
            facb = sm.tile([128, 1], fp)
            nc.scalar.copy(facb, fb_ps)
            w_n = sm.tile([128, 8], fp)
            nc.vector.tensor_scalar_mul(w_n, w_un, facb)
            wT_ps = mps.tile([8, 128], fp, tag="m", name="wT_ps")
            nc.tensor.transpose(wT_ps, w_n, ident)
            wT = sm.tile([8, 128], fp)
            nc.scalar.copy(wT, wT_ps)
            nc.gpsimd.dma_start(out=w_o.ap().rearrange("(t p) -> t p", t=8), in_=wT)

            # ---------- stage 8b: W_out c-phase ----------
            c_chunks = [(H + 128 * u, 128, (lambda u=u: jc[:, u : u + 1])) for u in range(8)]
            c_chunks.append((H + 1024, 8, lambda: jc[0:8, 8:9]))
            w_phase(c_chunks, first=False, last=True)

            # ---------- stage 9: evacuate logits ----------
            o_t = sm.tile([128, 5 * 512], fp)
            for g in range(5):
                nc.vector.tensor_add(
                    o_t[:, 512 * g : 512 * (g + 1)], pst[g], b_sb[:, 512 * g : 512 * (g + 1)]
                )
            for vb in range(NVB):
                n = _vb_n(vb)
                g, row = vb // 3, 32 * (vb % 3)
                nc.gpsimd.dma_start(
                    out=logits_o.ap()[512 * vb : 512 * vb + n].rearrange("(one n) -> one n", one=1),
                    in_=o_t[row : row + 1, 512 * g : 512 * g + n],
                )
            ring_ctx.__exit__(None, None, None)

    nc.compile()
    return nc


def _host_prep(word_input, last_context, prev_hidden, encoder_outputs, emb,
               W_ih, W_hh, b_ih, b_hh, W_a, b_a, W_out, b_out):
    word_input = np.asarray(word_input)
    x = np.concatenate(
        [np.asarray(emb)[int(word_input.reshape(-1)[0])].astype(F32),
         np.asarray(last_context, dtype=F32).reshape(H)]
    )  # [2064]
    hp = np.asarray(prev_hidden, dtype=F32).reshape(H)

    x_cols = np.zeros((128, 17), F32)
    x_cols[:, 0:16] = x[0:2048].reshape(16, 128).T
    x_cols[0:16, 16] = x[2048:2064]
    x_cols[16, 16] = 1.0  # bias fold
    hp_cols = np.zeros((128, 9), F32)
    hp_cols[:, 0:8] = hp[0:1024].reshape(8, 128).T
    hp_cols[0:8, 8] = hp[1024:1032]
    hp_cols[8, 8] = 1.0
    hp829 = np.ascontiguousarray(hp.reshape(8, HC))

    # permutation of h indices matching the on-chip column layout
    r = np.arange(1024)
    perm_h = np.concatenate([129 * (r // 128) + (r % 128), 129 * np.arange(8) + 128])
    perm_j = np.concatenate([perm_h, H + np.arange(H)])

    W_ih = np.asarray(W_ih, dtype=F32)
    W_hh = np.asarray(W_hh, dtype=F32)
    b_ih = np.asarray(b_ih, dtype=F32)
    b_hh = np.asarray(b_hh, dtype=F32)
    W_a = np.asarray(W_a, dtype=F32)
    W_out = np.asarray(W_out, dtype=F32)
    b_out = np.asarray(b_out, dtype=F32)
    enc = np.asarray(encoder_outputs, dtype=F32).reshape(S, H)

    wa_perm = np.ascontiguousarray(W_a[perm_h, :])

    in_maps = []
    for i in range(NCORES):
        gcols = np.concatenate([H * k + HC * i + np.arange(HC) for k in range(3)])
        wih_core = np.empty((IN + 1, GC), F32)
        wih_core[0:IN] = W_ih[gcols, :].T
        wih_core[IN] = b_ih[gcols]
        whh_core = np.empty((H + 1, GC), F32)
        whh_core[0:H] = W_hh[gcols, :].T
        whh_core[H] = b_hh[gcols]

        v0, rc = V0[i], VROWS[i]
        wpad = np.zeros((VC, IN), F32)
        wpad[0:rc] = W_out[v0 : v0 + rc]
        wt = np.ascontiguousarray(wpad.T)[perm_j]  # [2064, VC]

        b_core = np.zeros((128, 5 * 512), F32)
        bpad = np.zeros(VC, F32)
        bpad[0:rc] = b_out[v0 : v0 + rc]
        for vb in range(NVB):
            n = _vb_n(vb)
            b_core[32 * (vb % 3), 512 * (vb // 3) : 512 * (vb // 3) + n] = bpad[
                512 * vb : 512 * vb + n
            ]

        ohv = np.zeros((8, 1), F32)
        ohv[i, 0] = 1.0

        in_maps.append({
            "wih_t": wih_core,
            "whh_t": whh_core,
            "wa_p": wa_perm,
            "enc_c": np.ascontiguousarray(enc[SEQC * i : SEQC * (i + 1)]),
            "wout_t": wt,
            "b_c": b_core,
            "x_cols": x_cols,
            "hp_cols": hp_cols,
            "hp829": hp829,
            "oh": ohv,
        })
    return in_maps


def kernel(word_input, last_context, prev_hidden, encoder_outputs, emb,
           W_ih, W_hh, b_ih, b_hh, W_a, b_a, W_out, b_out):
    global LAST_EXEC_TIME_NS
    from concourse import bass_utils

    if "nc" not in _NC_CACHE:
        _NC_CACHE["nc"] = _build_nc()
    nc = _NC_CACHE["nc"]

    in_maps = _host_prep(word_input, last_context, prev_hidden, encoder_outputs, emb,
                         W_ih, W_hh, b_ih, b_hh, W_a, b_a, W_out, b_out)

    trace = os.environ.get("BASS_KERNEL_PROFILE", "") == "1"
    res = bass_utils.run_bass_kernel_spmd(
        nc, in_maps, core_ids=list(range(NCORES)), trace=trace
    )
    LAST_EXEC_TIME_NS = res.exec_time_ns

    logits = np.concatenate(
        [res.results[i]["logits_o"][0 : VROWS[i]] for i in range(NCORES)]
    )  # [V]
    z = logits.astype(np.float64)
    mx = z.max()
    lse = mx + np.log(np.exp(z - mx).sum())
    output = (z - lse).astype(F32).reshape(1, V)

    attn_context = res.results[0]["ctx_o"].reshape(1, 1, H)
    current_hidden = res.results[0]["h_o"].reshape(1, 1, H)
    attn_w = np.concatenate([res.results[i]["w_o"] for i in range(NCORES)]).reshape(1, 1, S)
    return output, attn_context, current_hidden, attn_w


# revision 7
# speedup vs baseline: 1.0789x; 1.0789x over previous
"""Trainium2 Bass kernel for nn_Match_Decoder (GRU step + Luong attention + vocab projection).

Strategy (8-core SPMD):
  - Host precomputes x = [emb[word], last_context] (embedding row gather) and
    transposes/permutes weights so every matvec runs on the PE with the big
    operand streaming as rhs from its natural DMA layout.
  - GRU: gx/gh row-sharded 387 rows/core (gate-aligned 129-chunks), AllGather
    (774 floats/core), every core computes gates redundantly on an [8,129] layout.
  - Attention: W_a replicated (rows permuted to match the h column layout), q and
    per-core partial softmax stats + context partials computed locally over a
    1024-step encoder shard; second AllGather (1034 floats/core) combines them.
  - Vocab projection: W_out columns permuted to the joined = [h; ctx] on-chip
    column layout, transposed to [2064, 6400] per core (vocab-sharded, padded),
    streamed as 18 row-chunk tiles; logits accumulate in PSUM as 13 x [1, <=512]
    rows packed 3-per-bank at partitions {0,32,64}.
  - Final log_softmax normalization on host (subtract scalar logsumexp).
"""

import os
import numpy as np

V = 50257
H = 1032
IN = 2 * H          # 2064
TH = 3 * H          # 3096
S = 8192
NCORES = 8
GC = TH // NCORES   # 387 gate rows per core
HC = H // NCORES    # 129
SEQC = S // NCORES  # 1024
VC = 6400           # padded vocab rows per core (50 tiles of 128)
NVB = 13            # vocab blocks: 12x512 + 1x256
VROWS = [6272] * 7 + [6353]
V0 = [6272 * i for i in range(7)] + [43904]

F32 = np.float32

LAST_EXEC_TIME_NS = None
_NC_CACHE = {}


def _vb_n(vb):
    return 512 if vb < 12 else 256


def _build_nc():
    import concourse.mybir as mybir
    import concourse.tile as tile
    from concourse import bacc
    from concourse.masks import make_identity

    fp = mybir.dt.float32
    nc = bacc.Bacc("TRN2", target_bir_lowering=False, debug=False, num_devices=NCORES)

    # ---- per-core DRAM inputs ----
    wih_t = nc.dram_tensor("wih_t", [IN + 1, GC], fp, kind="ExternalInput")
    whh_t = nc.dram_tensor("whh_t", [H + 1, GC], fp, kind="ExternalInput")
    wa_p = nc.dram_tensor("wa_p", [H, H], fp, kind="ExternalInput")
    enc_c = nc.dram_tensor("enc_c", [SEQC, H], fp, kind="ExternalInput")
    wout_t = nc.dram_tensor("wout_t", [IN, VC], fp, kind="ExternalInput")
    b_c = nc.dram_tensor("b_c", [128, 5 * 512], fp, kind="ExternalInput")
    x_cols = nc.dram_tensor("x_cols", [128, 17], fp, kind="ExternalInput")
    hp_cols = nc.dram_tensor("hp_cols", [128, 9], fp, kind="ExternalInput")
    hp829 = nc.dram_tensor("hp829", [8, HC], fp, kind="ExternalInput")
    oh = nc.dram_tensor("oh", [8, 1], fp, kind="ExternalInput")

    # ---- per-core DRAM outputs ----
    logits_o = nc.dram_tensor("logits_o", [VC], fp, kind="ExternalOutput")
    h_o = nc.dram_tensor("h_o", [H], fp, kind="ExternalOutput")
    ctx_o = nc.dram_tensor("ctx_o", [H], fp, kind="ExternalOutput")
    w_o = nc.dram_tensor("w_o", [SEQC], fp, kind="ExternalOutput")

    ax = mybir.AxisListType.X
    AF = mybir.ActivationFunctionType

    with tile.TileContext(nc) as tc:
        with (
            tc.tile_pool(name="const", bufs=1) as const,
            tc.tile_pool(name="sm", bufs=1) as sm,          # small long-lived sbuf
            tc.tile_pool(name="att", bufs=1) as att,        # W_a + enc
            tc.tile_pool(name="psw", bufs=1, space="PSUM") as psw,
            tc.tile_pool(name="mps", bufs=3, space="PSUM") as mps,
            tc.tile_pool(name="dram", bufs=1, space="DRAM") as dram,
        ):
            ident = const.tile([128, 128], fp)
            make_identity(nc, ident)
            ones = const.tile([128, 128], fp)
            nc.vector.memset(ones, 1.0)

            # PE warmup: ~40 quick matmuls so the HAM clock-gate opens before
            # the GRU matvecs arrive (otherwise they run at 1.2 GHz).
            warm_ps = mps.tile([1, 128], fp, tag="m", name="warm_ps")
            for _ in range(40):
                nc.tensor.matmul(warm_ps, ones[:, 0:1], ones[:, 0:128],
                                 start=True, stop=True)

            # ---------- stage 1: GRU matvecs gx, gh ----------
            x_sb = sm.tile([128, 17], fp)
            nc.sync.dma_start(out=x_sb, in_=x_cols.ap())
            hpc_sb = sm.tile([128, 9], fp)
            nc.sync.dma_start(out=hpc_sb, in_=hp_cols.ap())
            hp8_sb = sm.tile([8, HC], fp)
            nc.sync.dma_start(out=hp8_sb, in_=hp829.ap())
            oh_sb = sm.tile([8, 1], fp)
            nc.sync.dma_start(out=oh_sb, in_=oh.ap())
            b_sb = sm.tile([128, 5 * 512], fp)
            nc.gpsimd.dma_start(out=b_sb, in_=b_c.ap())

            gx_ps = mps.tile([1, 512], fp, tag="m", name="gx_ps")
            gh_ps = mps.tile([1, 512], fp, tag="m", name="gh_ps")
            with tc.tile_pool(name="gru", bufs=1) as gru:
                wih_sb = gru.tile([128, 16 * GC], fp)
                for hh in range(2):
                    nc.sync.dma_start(
                        out=wih_sb[:, 8 * GC * hh : 8 * GC * (hh + 1)].rearrange(
                            "p (t r) -> p t r", t=8),
                        in_=wih_t.ap()[1024 * hh : 1024 * (hh + 1), :].rearrange(
                            "(t p) r -> p t r", p=128)
                    )
                wih_st = gru.tile([17, GC], fp)
                nc.sync.dma_start(out=wih_st, in_=wih_t.ap()[2048 : IN + 1, :])
                whh_sb = gru.tile([128, 8 * GC], fp)
                for hh in range(2):
                    nc.sync.dma_start(
                        out=whh_sb[:, 4 * GC * hh : 4 * GC * (hh + 1)].rearrange(
                            "p (t r) -> p t r", t=4),
                        in_=whh_t.ap()[512 * hh : 512 * (hh + 1), :].rearrange(
                            "(t p) r -> p t r", p=128)
                    )
                whh_st = gru.tile([9, GC], fp)
                nc.sync.dma_start(out=whh_st, in_=whh_t.ap()[1024 : H + 1, :])

                for t in range(16):
                    nc.tensor.matmul(
                        gx_ps[0:1, 0:GC], x_sb[:, t : t + 1],
                        wih_sb[:, GC * t : GC * (t + 1)], start=(t == 0), stop=False,
                    )
                nc.tensor.matmul(
                    gx_ps[0:1, 0:GC], x_sb[0:17, 16:17], wih_st[0:17, :], start=False, stop=True
                )
                for t in range(8):
                    nc.tensor.matmul(
                        gh_ps[0:1, 0:GC], hpc_sb[:, t : t + 1],
                        whh_sb[:, GC * t : GC * (t + 1)], start=(t == 0), stop=False,
                    )
                nc.tensor.matmul(
                    gh_ps[0:1, 0:GC], hpc_sb[0:9, 8:9], whh_st[0:9, :], start=False, stop=True
                )
            pay1 = sm.tile([1, 2 * GC], fp)
            nc.scalar.copy(pay1[0:1, 0:GC], gx_ps[0:1, 0:GC])
            nc.scalar.copy(pay1[0:1, GC : 2 * GC], gh_ps[0:1, 0:GC])

            cc1_in = dram.tile([1, 2 * GC], fp)
            cc1_out = dram.tile([NCORES, 2 * GC], fp)
            nc.gpsimd.dma_start(out=cc1_in, in_=pay1)
            nc.gpsimd.collective_compute(
                "AllGather", mybir.AluOpType.bypass,
                replica_groups=[list(range(NCORES))],
                ins=[cc1_in.opt()], outs=[cc1_out.opt()],
            )
            g1 = sm.tile([8, 2 * GC], fp)
            nc.gpsimd.dma_start(out=g1, in_=cc1_out)

            # ---------- stage 2: gates on [8, 129] ----------
            xr, xz, xn = (g1[:, 129 * k : 129 * (k + 1)] for k in range(3))
            hr, hz, hn = (g1[:, GC + 129 * k : GC + 129 * (k + 1)] for k in range(3))
            t_r = sm.tile([8, HC], fp)
            nc.vector.tensor_add(t_r, xr, hr)
            r_g = sm.tile([8, HC], fp)
            nc.scalar.activation(r_g, t_r, AF.Sigmoid)
            t_z = sm.tile([8, HC], fp)
            nc.vector.tensor_add(t_z, xz, hz)
            z_g = sm.tile([8, HC], fp)
            nc.scalar.activation(z_g, t_z, AF.Sigmoid)
            t_n = sm.tile([8, HC], fp)
            nc.vector.tensor_mul(t_n, r_g, hn)
            nc.vector.tensor_add(t_n, t_n, xn)
            n_g = sm.tile([8, HC], fp)
            nc.scalar.activation(n_g, t_n, AF.Tanh)
            # h_new = n + z*(hp - n)
            h_new = sm.tile([8, HC], fp)
            nc.vector.tensor_sub(h_new, hp8_sb, n_g)
            nc.vector.tensor_mul(h_new, z_g, h_new)
            nc.vector.tensor_add(h_new, n_g, h_new)
            nc.gpsimd.dma_start(out=h_o.ap().rearrange("(i j) -> i j", i=8), in_=h_new)

            # ---------- stage 3: J_h columns ----------
            hT_ps = mps.tile([128, 8], fp, tag="m", name="hT_ps")
            nc.tensor.transpose(hT_ps, h_new[:, 0:128], ident[0:8, 0:8])
            hT = sm.tile([128, 8], fp)
            nc.scalar.copy(hT, hT_ps)
            h_strag = h_new[0:8, 128:129]  # K=8 column

            # ---------- stage 4: q = h @ W_a (permuted rows) ----------
            wa_sb = att.tile([128, 8 * H], fp)
            nc.sync.dma_start(
                out=wa_sb.rearrange("p (t l) -> p t l", t=8),
                in_=wa_p.ap()[0:1024, :].rearrange("(t p) l -> p t l", p=128)
            )
            wa_st = att.tile([8, H], fp)
            nc.sync.dma_start(out=wa_st, in_=wa_p.ap()[1024:H, :])

            q_row = sm.tile([1, H], fp)
            for c0, nn in ((0, 512), (512, 512), (1024, 8)):
                qp = mps.tile([1, 512], fp, tag="m", name=f"qp{c0}")
                for m in range(8):
                    nc.tensor.matmul(
                        qp[0:1, 0:nn], hT[:, m : m + 1],
                        wa_sb[:, H * m + c0 : H * m + c0 + nn], start=(m == 0), stop=False,
                    )
                nc.tensor.matmul(
                    qp[0:1, 0:nn], h_strag, wa_st[0:8, c0 : c0 + nn], start=False, stop=True
                )
                nc.scalar.copy(q_row[0:1, c0 : c0 + nn], qp[0:1, 0:nn])
            q_rep = sm.tile([128, H], fp)
            for c0, nn in ((0, 512), (512, 512), (1024, 8)):
                qr_ps = mps.tile([128, 512], fp, tag="m", name=f"qr{c0}")
                nc.tensor.matmul(
                    qr_ps[:, 0:nn], ones[0:1, 0:128], q_row[0:1, c0 : c0 + nn],
                    start=True, stop=True,
                )
                nc.scalar.copy(q_rep[:, c0 : c0 + nn], qr_ps[:, 0:nn])

            # ---------- stage 5: scores + local softmax stats ----------
            enc_sb = att.tile([128, 8 * H], fp)
            nc.sync.dma_start(
                out=enc_sb.rearrange("p (t l) -> p t l", t=8),
                in_=enc_c.ap().rearrange("(t p) l -> p t l", p=128)
            )
            s_sb = sm.tile([128, 8], fp)
            tmp = sm.tile([128, H], fp)
            for t in range(8):
                nc.vector.tensor_mul(tmp, enc_sb[:, H * t : H * (t + 1)], q_rep)
                nc.vector.reduce_sum(s_sb[:, t : t + 1], tmp, axis=ax)
            m_p = sm.tile([128, 1], fp)
            nc.vector.reduce_max(m_p, s_sb, axis=ax)
            mT_ps = mps.tile([1, 128], fp, tag="m", name="mT_ps")
            nc.tensor.transpose(mT_ps, m_p, ident)
            mT = sm.tile([1, 128], fp)
            nc.scalar.copy(mT, mT_ps)
            m_i = sm.tile([1, 1], fp)
            nc.vector.reduce_max(m_i, mT, axis=ax)
            negm = sm.tile([1, 1], fp)
            nc.vector.tensor_scalar_mul(negm, m_i, -1.0)
            nb_ps = mps.tile([128, 1], fp, tag="m", name="nb_ps")
            nc.tensor.matmul(nb_ps, ones[0:1, 0:128], negm, start=True, stop=True)
            negb = sm.tile([128, 1], fp)
            nc.scalar.copy(negb, nb_ps)
            w_un = sm.tile([128, 8], fp)
            e_p = sm.tile([128, 1], fp)
            nc.scalar.activation(w_un, s_sb, AF.Exp, bias=negb, accum_out=e_p)
            ei_ps = mps.tile([1, 1], fp, tag="m", name="ei_ps")
            nc.tensor.matmul(ei_ps, ones[0:128, 0:1], e_p, start=True, stop=True)

            # ---------- stage 6: context partial + collective 2 ----------
            pay2 = sm.tile([1, H + 2], fp)
            for c0, nn in ((0, 512), (512, 512), (1024, 8)):
                cp = mps.tile([1, 512], fp, tag="m", name=f"cp{c0}")
                for t in range(8):
                    nc.tensor.matmul(
                        cp[0:1, 0:nn], w_un[:, t : t + 1],
                        enc_sb[:, H * t + c0 : H * t + c0 + nn],
                        start=(t == 0), stop=(t == 7),
                    )
                nc.scalar.copy(pay2[0:1, c0 : c0 + nn], cp[0:1, 0:nn])
            nc.scalar.copy(pay2[0:1, H : H + 1], m_i)
            nc.scalar.copy(pay2[0:1, H + 1 : H + 2], ei_ps)

            cc2_in = dram.tile([1, H + 2], fp)
            cc2_out = dram.tile([NCORES, H + 2], fp)
            nc.gpsimd.dma_start(out=cc2_in, in_=pay2)
            nc.gpsimd.collective_compute(
                "AllGather", mybir.AluOpType.bypass,
                replica_groups=[list(range(NCORES))],
                ins=[cc2_in.opt()], outs=[cc2_out.opt()],
            )
            g2 = sm.tile([8, H + 2], fp)
            nc.gpsimd.dma_start(out=g2, in_=cc2_out)

            # ---------- stage 8a: W_out h-phase (emitted before combine for PE order) ----------
            pst = [psw.tile([128, 512], fp, tag=f"ps{g}", name=f"ps{g}") for g in range(5)]
            ring_ctx = tc.tile_pool(name="ring", bufs=3)
            ring = ring_ctx.__enter__()

            def w_phase(chunks, first, last):
                for t, (r0, k, jcol) in enumerate(chunks):
                    wt = ring.tile([128, VC], fp, tag="w", name=f"wt{r0}")
                    nc.sync.dma_start(out=wt[0:k, :], in_=wout_t.ap()[r0 : r0 + k, :])
                    for vb in range(NVB):
                        n = _vb_n(vb)
                        pt, row = pst[vb // 3], 32 * (vb % 3)
                        nc.tensor.matmul(
                            pt[row : row + 1, 0:n], jcol()[0:k, :],
                            wt[0:k, 512 * vb : 512 * vb + n],
                            start=(first and t == 0), stop=(last and t == len(chunks) - 1),
                        )

            h_chunks = [(128 * t, 128, (lambda t=t: hT[:, t : t + 1])) for t in range(8)]
            h_chunks.append((1024, 8, lambda: h_strag))
            w_phase(h_chunks, first=True, last=False)

            # ---------- stage 7: cross-core softmax combine ----------
            mcol = g2[0:8, H : H + 1]
            ecol = g2[0:8, H + 1 : H + 2]
            mT2_ps = mps.tile([1, 8], fp, tag="m", name="mT2_ps")
            nc.tensor.transpose(mT2_ps, mcol, ident[0:8, 0:8])
            mT2 = sm.tile([1, 8], fp)
            nc.scalar.copy(mT2, mT2_ps)
            M_i = sm.tile([1, 1], fp)
            nc.vector.reduce_max(M_i, mT2, axis=ax)
            negM = sm.tile([1, 1], fp)
            nc.vector.tensor_scalar_mul(negM, M_i, -1.0)
            n8_ps = mps.tile([8, 1], fp, tag="m", name="n8_ps")
            nc.tensor.matmul(n8_ps, ones[0:1, 0:8], negM, start=True, stop=True)
            negM8 = sm.tile([8, 1], fp)
            nc.scalar.copy(negM8, n8_ps)
            expm = sm.tile([8, 1], fp)
            nc.scalar.activation(expm, mcol, AF.Exp, bias=negM8)
            prod = sm.tile([8, 1], fp)
            nc.vector.tensor_mul(prod, ecol, expm)
            tot_ps = mps.tile([1, 1], fp, tag="m", name="tot_ps")
            nc.tensor.matmul(tot_ps, ones[0:8, 0:1], prod, start=True, stop=True)
            tot = sm.tile([1, 1], fp)
            nc.scalar.copy(tot, tot_ps)
            invt = sm.tile([1, 1], fp)
            nc.vector.reciprocal(invt, tot)
            scaled = sm.tile([8, H], fp)
            nc.vector.tensor_scalar_mul(scaled, g2[0:8, 0:H], expm)
            ctx_un = sm.tile([1, H], fp)
            for c0, nn in ((0, 512), (512, 512), (1024, 8)):
                cr_ps = mps.tile([1, 512], fp, tag="m", name=f"cr{c0}")
                nc.tensor.matmul(
                    cr_ps[0:1, 0:nn], ones[0:8, 0:1], scaled[:, c0 : c0 + nn],
                    start=True, stop=True,
                )
                nc.scalar.copy(ctx_un[0:1, c0 : c0 + nn], cr_ps[0:1, 0:nn])
            ctx_row = sm.tile([1, H], fp)
            nc.vector.tensor_scalar_mul(ctx_row, ctx_un, invt)
            nc.gpsimd.dma_start(out=ctx_o.ap().rearrange("(one h) -> one h", one=1), in_=ctx_row)

            # J_c columns via transposes of ctx_row
            jc_ps = mps.tile([128, 9], fp, tag="m", name="jc_ps")
            for u in range(8):
                nc.tensor.transpose(
                    jc_ps[:, u : u + 1], ctx_row[0:1, 128 * u : 128 * (u + 1)],
                    ident[0:1, 0:1],
                )
            nc.tensor.transpose(jc_ps[0:8, 8:9], ctx_row[0:1, 1024:H], ident[0:1, 0:1])
            jc = sm.tile([128, 9], fp)
            nc.scalar.copy(jc, jc_ps)

            # attn weights normalize + output
            me_ps = mps.tile([1, 1], fp, tag="m", name="me_ps")
            nc.tensor.matmul(me_ps, oh_sb, expm, start=True, stop=True)
            me = sm.tile([1, 1], fp)
            nc.scalar.copy(me, me_ps)
            fac = sm.tile([1, 1], fp)
            nc.vector.tensor_mul(fac, me, invt)
            fb_ps = mps.tile([128, 1], fp, tag="m", name="fb_ps")
            nc.tensor.matmul(fb_ps, ones[0:1, 0:128], fac, start=True, stop=True)
            facb = sm.tile([128, 1], fp)
            nc.scalar.copy(facb, fb_ps)
            w_n = sm.tile([128, 8], fp)
            nc.vector.tensor_scalar_mul(w_n, w_un, facb)
            wT_ps = mps.tile([8, 128], fp, tag="m", name="wT_ps")
            nc.tensor.transpose(wT_ps, w_n, ident)
            wT = sm.tile([8, 128], fp)
            nc.scalar.copy(wT, wT_ps)
            nc.gpsimd.dma_start(out=w_o.ap().rearrange("(t p) -> t p", t=8), in_=wT)

            # ---------- stage 8b: W_out c-phase ----------
            c_chunks = [(H + 128 * u, 128, (lambda u=u: jc[:, u : u + 1])) for u in range(8)]
            c_chunks.append((H + 1024, 8, lambda: jc[0:8, 8:9]))
            w_phase(c_chunks, first=False, last=True)

            # ---------- stage 9: evacuate logits ----------
            o_t = b_sb  # in-place: logits = psum + b
            for g in range(5):
                nc.vector.tensor_add(
                    o_t[:, 512 * g : 512 * (g + 1)], pst[g], b_sb[:, 512 * g : 512 * (g + 1)]
                )
            for vb in range(NVB):
                n = _vb_n(vb)
                g, row = vb // 3, 32 * (vb % 3)
                nc.gpsimd.dma_start(
                    out=logits_o.ap()[512 * vb : 512 * vb + n].rearrange("(one n) -> one n", one=1),
                    in_=o_t[row : row + 1, 512 * g : 512 * g + n],
                )
            ring_ctx.__exit__(None, None, None)

    nc.compile()
    return nc


def _host_prep(word_input, last_context, prev_hidden, encoder_outputs, emb,
               W_ih, W_hh, b_ih, b_hh, W_a, b_a, W_out, b_out):
    word_input = np.asarray(word_input)
    x = np.concatenate(
        [np.asarray(emb)[int(word_input.reshape(-1)[0])].astype(F32),
         np.asarray(last_context, dtype=F32).reshape(H)]
    )  # [2064]
    hp = np.asarray(prev_hidden, dtype=F32).reshape(H)

    x_cols = np.zeros((128, 17), F32)
    x_cols[:, 0:16] = x[0:2048].reshape(16, 128).T
    x_cols[0:16, 16] = x[2048:2064]
    x_cols[16, 16] = 1.0  # bias fold
    hp_cols = np.zeros((128, 9), F32)
    hp_cols[:, 0:8] = hp[0:1024].reshape(8, 128).T
    hp_cols[0:8, 8] = hp[1024:1032]
    hp_cols[8, 8] = 1.0
    hp829 = np.ascontiguousarray(hp.reshape(8, HC))

    # permutation of h indices matching the on-chip column layout
    r = np.arange(1024)
    perm_h = np.concatenate([129 * (r // 128) + (r % 128), 129 * np.arange(8) + 128])
    perm_j = np.concatenate([perm_h, H + np.arange(H)])

    W_ih = np.asarray(W_ih, dtype=F32)
    W_hh = np.asarray(W_hh, dtype=F32)
    b_ih = np.asarray(b_ih, dtype=F32)
    b_hh = np.asarray(b_hh, dtype=F32)
    W_a = np.asarray(W_a, dtype=F32)
    W_out = np.asarray(W_out, dtype=F32)
    b_out = np.asarray(b_out, dtype=F32)
    enc = np.asarray(encoder_outputs, dtype=F32).reshape(S, H)

    wa_perm = np.ascontiguousarray(W_a[perm_h, :])

    in_maps = []
    for i in range(NCORES):
        gcols = np.concatenate([H * k + HC * i + np.arange(HC) for k in range(3)])
        wih_core = np.empty((IN + 1, GC), F32)
        wih_core[0:IN] = W_ih[gcols, :].T
        wih_core[IN] = b_ih[gcols]
        whh_core = np.empty((H + 1, GC), F32)
        whh_core[0:H] = W_hh[gcols, :].T
        whh_core[H] = b_hh[gcols]

        v0, rc = V0[i], VROWS[i]
        wpad = np.zeros((VC, IN), F32)
        wpad[0:rc] = W_out[v0 : v0 + rc]
        wt = np.ascontiguousarray(wpad.T)[perm_j]  # [2064, VC]

        b_core = np.zeros((128, 5 * 512), F32)
        bpad = np.zeros(VC, F32)
        bpad[0:rc] = b_out[v0 : v0 + rc]
        for vb in range(NVB):
            n = _vb_n(vb)
            b_core[32 * (vb % 3), 512 * (vb // 3) : 512 * (vb // 3) + n] = bpad[
                512 * vb : 512 * vb + n
            ]

        ohv = np.zeros((8, 1), F32)
        ohv[i, 0] = 1.0

        in_maps.append({
            "wih_t": wih_core,
            "whh_t": whh_core,
            "wa_p": wa_perm,
            "enc_c": np.ascontiguousarray(enc[SEQC * i : SEQC * (i + 1)]),
            "wout_t": wt,
            "b_c": b_core,
            "x_cols": x_cols,
            "hp_cols": hp_cols,
            "hp829": hp829,
            "oh": ohv,
        })
    return in_maps


def kernel(word_input, last_context, prev_hidden, encoder_outputs, emb,
           W_ih, W_hh, b_ih, b_hh, W_a, b_a, W_out, b_out):
    global LAST_EXEC_TIME_NS
    from concourse import bass_utils

    if "nc" not in _NC_CACHE:
        _NC_CACHE["nc"] = _build_nc()
    nc = _NC_CACHE["nc"]

    in_maps = _host_prep(word_input, last_context, prev_hidden, encoder_outputs, emb,
                         W_ih, W_hh, b_ih, b_hh, W_a, b_a, W_out, b_out)

    trace = os.environ.get("BASS_KERNEL_PROFILE", "") == "1"
    res = bass_utils.run_bass_kernel_spmd(
        nc, in_maps, core_ids=list(range(NCORES)), trace=trace
    )
    LAST_EXEC_TIME_NS = res.exec_time_ns

    logits = np.concatenate(
        [res.results[i]["logits_o"][0 : VROWS[i]] for i in range(NCORES)]
    )  # [V]
    z = logits.astype(np.float64)
    mx = z.max()
    lse = mx + np.log(np.exp(z - mx).sum())
    output = (z - lse).astype(F32).reshape(1, V)

    attn_context = res.results[0]["ctx_o"].reshape(1, 1, H)
    current_hidden = res.results[0]["h_o"].reshape(1, 1, H)
    attn_w = np.concatenate([res.results[i]["w_o"] for i in range(NCORES)]).reshape(1, 1, S)
    return output, attn_context, current_hidden, attn_w
